# revision 6
# baseline (speedup 1.0000x reference)
"""Masked multi-head self-attention for 8 Trainium2 NeuronCores.

Full module: qkv projection -> causal softmax attention (16 heads) -> out
projection, x[4, 2048, 1024].  Core c: batch c//2, heads (c%2)*8 .. +8.

v2 structure:
- Attention runs in PAIR-UNITS (qb, j) covering heads 2j, 2j+1.  Score
  matmuls contract K=64; the two heads of a pair live at SBUF partitions
  0-63 / 64-127 of one Q/K chunk, so they issue as row-tiled PE matmuls
  (tile_position (0,0) / (64,0)) that execute CONCURRENTLY on the
  array -- ~2x score throughput vs serial.
- K-projection bias dropped exactly (softmax shift invariance); V bias
  folded into the output bias on the host; Q bias applied by the vector
  engine.  The scalar engine runs ONLY softmax exp (the co-critical
  resource at ~172us).
- Out-projection is LOCAL: each core contracts its 512 attention
  channels against all 1024 w_out columns -> partial [1024 oc, 512 t]
  per query block; a pairwise ReduceScatter(add) produces each core's
  final half directly.  No AllGather, no gathered-weight loads, no
  cross-core dependency in the matmul stream.
- One software pipeline: stage-1 quanta (Q/K t-halves, V tile-pairs,
  ~3.4us each) and out-projection blocks are drawn as ATOMIC fillers
  between score tiles so the tensor queue never idles while the scalar
  engine chews exp.  Quanta are atomic because a partially-emitted
  quantum holding a rotating PSUM slot would head-of-line deadlock the
  in-order tensor queue.
"""

import math
import os
import sys

for _p in ("/opt/trn_rl_repo", "/root/.axon_site/_ro/trn_rl_repo"):
    if os.path.isdir(_p) and _p not in sys.path:
        sys.path.insert(0, _p)
        break

import ml_dtypes
import numpy as np

import concourse.bass as bass
import concourse.mybir as mybir
import concourse.tile as tile
from concourse import bacc
from concourse.bass_utils import run_bass_kernel_spmd

B, T, C, H = 4, 2048, 1024, 16
D = 64
NCORES = 8
HPC = 8                # heads per core
NPAIR = 4              # head pairs per core
CPC = 512              # channels per core
P = 128
QB = 512
NQB = 4
KC = 8                 # 128-row contraction chunks of C
SCALE = 1.0 / math.sqrt(D)

F32 = mybir.dt.float32
BF16 = mybir.dt.bfloat16
EXP = mybir.ActivationFunctionType.Exp

_CACHE = {}


def build():
    nc = bacc.Bacc("TRN2", num_devices=NCORES, debug=False)

    x_r = nc.dram_tensor("x_r", [P, KC * T], BF16, kind="ExternalInput")
    wqkv_r = nc.dram_tensor(
        "wqkv_r", [P, 12 * KC * P], BF16, kind="ExternalInput"
    )
    wo_r = nc.dram_tensor("wo_r", [P, NPAIR * 8 * P], BF16, kind="ExternalInput")
    bq_r = nc.dram_tensor("bq_r", [P, 4], F32, kind="ExternalInput")
    bo_r = nc.dram_tensor("bo_r", [P, 8], F32, kind="ExternalInput")
    outT = nc.dram_tensor("outT", [CPC, T], BF16, kind="ExternalOutput")

    groups = [[0, 1], [2, 3], [4, 5], [6, 7]]

    with tile.TileContext(nc) as tc:
        with (
            tc.tile_pool(name="const", bufs=1) as constp,
            tc.tile_pool(name="xw", bufs=1) as xwp,
            tc.tile_pool(name="yt", bufs=1) as ytp,
            tc.tile_pool(name="va", bufs=1) as vap,
            tc.tile_pool(name="pt", bufs=24) as ptp,
            tc.tile_pool(name="atv", bufs=1) as atvp,
            tc.tile_pool(name="rc", bufs=2) as rcp,
            tc.tile_pool(name="bc", bufs=2) as bcp,
            tc.tile_pool(name="osb", bufs=1) as osbp,
            tc.tile_pool(name="dram", bufs=1, space="DRAM") as dramp,
            tc.tile_pool(name="scp", bufs=3, space="PSUM") as scp,
            tc.tile_pool(name="pap", bufs=1, space="PSUM") as pap,
        ):
            bq_sb = constp.tile([P, 4], F32, tag="bq")
            bo_sb = constp.tile([P, 8], F32, tag="bo")

            xt = xwp.tile([P, KC * T], BF16, tag="xt")
            xt3 = xt[:].rearrange("p (k t) -> p k t", k=KC)
            w3t = xwp.tile([P, 12 * KC * P], BF16, tag="w3")
            w3 = w3t[:].rearrange("p (n k c) -> p n k c", n=12, k=KC)
            w2t = xwp.tile([P, NPAIR * 8 * P], BF16, tag="w2")
            w2 = w2t[:].rearrange("p (j o c) -> p j o c", j=NPAIR, o=8)

            yts = [
                ytp.tile([P, T], BF16, name=f"yt{n}", tag=f"yt{n}")
                for n in range(8)
            ]
            vaug = vap.tile([P, HPC * 16 * 65], BF16, tag="vaug")
            vaug4 = vaug[:].rearrange("p (h k c) -> p h k c", h=HPC, c=65)
            nc.vector.memset(vaug4[:, :, :, 64:65], 1.0)

            # ---------------- startup DMAs ----------------
            h0 = slice(0, T // 2)
            h1 = slice(T // 2, T)
            xsrc = x_r.ap().rearrange("p (k t) -> p k t", k=KC)
            wsrc = wqkv_r.ap().rearrange("p (n k c) -> p n k c", n=12, k=KC)

            dq = [nc.sync, nc.scalar, nc.gpsimd]
            dqi = [0]

            def dma(dst, src):
                dq[dqi[0] % 3].dma_start(dst, src)
                dqi[0] += 1

            dma(w3[:, 4:5, :, :], wsrc[:, 4:5, :, :])          # K0
            for kk in range(0, KC, 2):                          # x h0
                dma(xt3[:, kk:kk + 2, h0], xsrc[:, kk:kk + 2, h0])
            dma(w3[:, 8:10, :, :], wsrc[:, 8:10, :, :])         # V01
            dma(w3[:, 0:1, :, :], wsrc[:, 0:1, :, :])           # Q0
            dma(w3[:, 10:12, :, :], wsrc[:, 10:12, :, :])       # V23
            dma(bq_sb[:], bq_r.ap())
            dma(bo_sb[:], bo_r.ap())
            dma(w3[:, 5:6, :, :], wsrc[:, 5:6, :, :])           # K1
            dma(w3[:, 1:2, :, :], wsrc[:, 1:2, :, :])           # Q1
            dma(w3[:, 6:8, :, :], wsrc[:, 6:8, :, :])           # K23
            dma(w3[:, 2:4, :, :], wsrc[:, 2:4, :, :])           # Q23
            for kk in range(0, KC, 2):                          # x h1
                dma(xt3[:, kk:kk + 2, h1], xsrc[:, kk:kk + 2, h1])
            dma(w2t[:], wo_r.ap())

            # ---------------- stage-1 quanta (atomic closures) --------
            def qk_half(n, half):
                def run():
                    py = scp.tile(
                        [P, 2 * QB], F32, tag="ps", name=f"s1_{n}_{half}"
                    )
                    for i in range(2):
                        tc4 = 2 * half + i
                        for kc in range(KC):
                            nc.tensor.matmul(
                                py[:, i * QB:(i + 1) * QB],
                                w3[:, n, kc, :],
                                xt3[:, kc, tc4 * QB:(tc4 + 1) * QB],
                                start=(kc == 0),
                                stop=(kc == KC - 1),
                            )
                    dst = yts[n][:, half * 2 * QB:(half + 1) * 2 * QB]
                    if n < 4:
                        nc.vector.tensor_scalar_add(
                            dst, py[:], bq_sb[:, n:n + 1]
                        )
                    else:
                        nc.vector.tensor_copy(dst, py[:])
                return run

            def v_pair(tp):
                def run():
                    pv = scp.tile([P, 2 * QB], F32, tag="ps", name=f"v{tp}")
                    for i in range(2):
                        tt = 2 * tp + i
                        for kc in range(KC):
                            nc.tensor.matmul(
                                pv[:, i * QB:(i + 1) * QB],
                                xt3[:, kc, tt * P:(tt + 1) * P],
                                w3[:, 8:12, kc, :],
                                start=(kc == 0),
                                stop=(kc == KC - 1),
                            )
                    nc.vector.tensor_copy(
                        vaug4[:, :, 2 * tp:2 * tp + 2, 0:64],
                        pv[:].rearrange("p (t h d) -> p h t d", t=2, d=64),
                    )
                return run

            # named deck of atomic fillers, drawn in order between score
            # tiles; ensure() force-runs named prerequisites
            deck = [
                ("V2", v_pair(2)), ("V3", v_pair(3)),
                ("K1h0", qk_half(5, 0)), ("Q1h0", qk_half(1, 0)),
                ("K2h0", qk_half(6, 0)), ("Q2h0", qk_half(2, 0)),
                ("K3h0", qk_half(7, 0)), ("Q3h0", qk_half(3, 0)),
                ("V4", v_pair(4)), ("V5", v_pair(5)),
                ("V6", v_pair(6)), ("V7", v_pair(7)),
                ("K0h1", qk_half(4, 1)), ("Q0h1", qk_half(0, 1)),
                ("K1h1", qk_half(5, 1)), ("Q1h1", qk_half(1, 1)),
                ("K2h1", qk_half(6, 1)), ("Q2h1", qk_half(2, 1)),
                ("K3h1", qk_half(7, 1)), ("Q3h1", qk_half(3, 1)),
            ]
            pend = []          # out-projection step queues (lists)
            rr = [0]

            def draw(n):
                for _ in range(n):
                    # round-robin between pending oproj steps and deck
                    use_pend = pend and (rr[0] % 2 == 0 or not deck)
                    rr[0] += 1
                    if use_pend:
                        pend[0].pop(0)()
                        if not pend[0]:
                            pend.pop(0)
                    elif deck:
                        deck.pop(0)[1]()
                    elif pend:
                        pend[0].pop(0)()
                        if not pend[0]:
                            pend.pop(0)
                    else:
                        return

            def ensure(names):
                for nm in names:
                    for i, (dn, fn) in enumerate(deck):
                        if dn == nm:
                            deck.pop(i)[1]()
                            break

            # ---------------- attention pair-units ----------------
            def s_pass(qb, j, kt_order, filler):
                qt = yts[j]
                kt_c = yts[4 + j]
                out = []
                for ki, kt in enumerate(kt_order):
                    filler(ki)
                    diag = kt >= 4 * qb
                    qoff = (kt - 4 * qb) * P if diag else 0
                    ps = scp.tile([P, 2 * QB], F32, tag="ps")
                    pt = ptp.tile([P, 2 * QB], BF16, tag="pt")
                    for hh in range(2):
                        nc.tensor.matmul(
                            ps[:, hh * QB + qoff:(hh + 1) * QB],
                            kt_c[hh * 64:hh * 64 + 64, kt * P:(kt + 1) * P],
                            qt[hh * 64:hh * 64 + 64,
                               qb * QB + qoff:(qb + 1) * QB],
                            start=True, stop=True,
                            tile_position=(hh * 64, 0),
                        )
                    if diag:
                        for hh in range(2):
                            nc.scalar.activation(
                                pt[:, hh * QB + qoff:(hh + 1) * QB],
                                ps[:, hh * QB + qoff:(hh + 1) * QB],
                                EXP, scale=SCALE,
                            )
                            nc.gpsimd.affine_select(
                                out=pt[:, hh * QB + qoff:hh * QB + qoff + P],
                                in_=pt[:, hh * QB + qoff:hh * QB + qoff + P],
                                compare_op=mybir.AluOpType.is_ge,
                                fill=0.0,
                                base=0,
                                pattern=[[1, P]],
                                channel_multiplier=-1,
                            )
                    else:
                        nc.scalar.activation(pt[:], ps[:], EXP, scale=SCALE)
                    out.append((kt, qoff, pt))
                return out

            def make_pv(u, pairs):
                qb, j = u
                ensure([f"V{tp}" for tp in range(2 * qb + 2)])
                pa = pap.tile([P, 2 * QB], F32, tag="pa")
                seq = [
                    (kt, qoff, pt, hh)
                    for (kt, qoff, pt) in pairs
                    for hh in range(2)
                ]
                st = {"i": 0}

                def emit(k):
                    end = min(len(seq), st["i"] + k)
                    while st["i"] < end:
                        kt, qoff, pt, hh = seq[st["i"]]
                        nc.tensor.matmul(
                            pa[0:65, hh * QB + qoff:(hh + 1) * QB],
                            vaug4[:, 2 * j + hh, kt, :],
                            pt[:, hh * QB + qoff:(hh + 1) * QB],
                            start=(st["i"] < 2),
                            stop=(st["i"] >= len(seq) - 2),
                            skip_group_check=True,
                        )
                        st["i"] += 1

                def finish():
                    emit(len(seq))
                    return pa

                return emit, finish

            atv_t = {}

            def norm(u, pa):
                # recip input must sit at partition 0 and partition_broadcast
                # only writes base-0 outputs (HW-verified); shifted-out
                # vector writes are fine.
                qb, j = u
                sums = rcp.tile([1, 2 * QB], F32, tag="rc")
                nc.vector.tensor_copy(sums[:], pa[64:65, :])
                rec = rcp.tile([1, 2 * QB], F32, tag="rc")
                nc.vector.reciprocal_approx_fast(rec[:], sums[:])
                bcx = bcp.tile([64, 2 * QB], F32, tag="bc")
                nc.gpsimd.partition_broadcast(bcx[:, 0:QB], rec[0:1, 0:QB])
                nc.gpsimd.partition_broadcast(
                    bcx[:, QB:2 * QB], rec[0:1, QB:2 * QB]
                )
                atv = atvp.tile([P, QB], BF16, tag=f"atv{qb}_{j}")
                nc.vector.tensor_mul(
                    atv[0:64, :], pa[0:64, 0:QB], bcx[:, 0:QB]
                )
                nc.vector.tensor_mul(
                    atv[64:128, :], pa[0:64, QB:2 * QB], bcx[:, QB:2 * QB]
                )
                atv_t[(qb, j)] = atv

            # ---------------- out-projection + reduce-scatter ----------
            prt = [
                dramp.tile([8 * P, QB], BF16, name=f"prt{qb}", tag=f"prt{qb}")
                for qb in range(NQB)
            ]
            rso = [
                dramp.tile([CPC, QB], BF16, name=f"rso{qb}", tag=f"rso{qb}")
                for qb in range(NQB)
            ]

            def oproj_steps(qb):
                osb = osbp.tile([P, 8 * QB], BF16, tag="osb")
                steps = []

                def block(ob):
                    def run():
                        po = scp.tile([P, 2 * QB], F32, tag="ps")
                        for sub in range(2):
                            oc = 2 * ob + sub
                            for j in range(NPAIR):
                                nc.tensor.matmul(
                                    po[:, sub * QB:(sub + 1) * QB],
                                    w2[:, j, oc, :],
                                    atv_t[(qb, j)][:],
                                    start=(j == 0),
                                    stop=(j == NPAIR - 1),
                                )
                        for sub in range(2):
                            oc = 2 * ob + sub
                            nc.vector.tensor_scalar_add(
                                osb[:, oc * QB:(oc + 1) * QB],
                                po[:, sub * QB:(sub + 1) * QB],
                                bo_sb[:, oc:oc + 1],
                            )
                    return run

                for ob in range(4):
                    steps.append(block(ob))

                def fin():
                    nc.sync.dma_start(
                        prt[qb][:].rearrange("(o p) n -> p o n", p=P),
                        osb[:].rearrange("p (o n) -> p o n", n=QB),
                    )
                    nc.gpsimd.collective_compute(
                        "ReduceScatter",
                        mybir.AluOpType.add,
                        replica_groups=groups,
                        ins=[prt[qb][:].opt()],
                        outs=[rso[qb][:].opt()],
                    )
                    nc.sync.dma_start(
                        outT[:, qb * QB:(qb + 1) * QB], rso[qb][:]
                    )
                steps.append(fin)
                return steps

            # ---------------- merged schedule ----------------
            units = [
                (0, 0), (1, 0), (0, 1), (1, 1),
                (0, 2), (1, 2), (0, 3), (1, 3),
                (3, 0), (2, 0), (3, 1), (2, 1),
                (3, 2), (3, 3), (2, 2), (2, 3),
            ]
            prereq = {}
            for (qb, j) in units:
                need = [f"K{j}h0"] if qb <= 1 else [f"K{j}h0", f"K{j}h1"]
                need.append(f"Q{j}h0" if qb <= 1 else f"Q{j}h1")
                prereq[(qb, j)] = [
                    nm for nm in need
                    if nm not in ("K0h0", "Q0h0")  # runway
                ]
            quota = [2, 2, 2, 2, 2, 2, 2, 2, 5, 5, 4, 3, 2, 2, 2, 2]
            oproj_at = {8: 0, 9: 1, 15: 3}

            # runway
            for fn in (qk_half(4, 0), v_pair(0), v_pair(1), qk_half(0, 0)):
                fn()

            pipe = []
            for ui, u in enumerate(units):
                qb, j = u
                ensure(prereq[u])
                if ui in oproj_at:
                    pend.append(oproj_steps(oproj_at[ui]))
                if u == (3, 0):
                    kt_order = list(range(16))
                else:
                    kt_order = (
                        list(range(4 * qb, 4 * qb + 4))
                        + list(range(0, 4 * qb))
                    )
                nkt = len(kt_order)
                if pipe:
                    up, upairs = pipe.pop(0)
                    emit_pv, finish_pv = make_pv(up, upairs)
                    pv_per = -(-2 * len(upairs) // nkt)
                else:
                    emit_pv = finish_pv = None
                    pv_per = 0
                q = quota[ui]
                acc = {"d": 0}

                def filler(ki, nkt=nkt, q=q, acc=acc, emit_pv=emit_pv,
                           pv_per=pv_per):
                    if emit_pv is not None:
                        emit_pv(pv_per)
                    want = ((ki + 1) * q) // nkt
                    if want > acc["d"]:
                        draw(want - acc["d"])
                        acc["d"] = want

                pairs = s_pass(qb, j, kt_order, filler)
                if finish_pv is not None:
                    norm(up, finish_pv())
                pipe.append((u, pairs))

            # drain: pv+norm of the last unit, leftovers, qb2 out-proj
            for up, upairs in pipe:
                emit_pv, finish_pv = make_pv(up, upairs)
                norm(up, finish_pv())
            while deck or pend:
                draw(100)
            for step in oproj_steps(2):
                step()

    nc.compile()
    return nc


def kernel(x, w_qkv, b_qkv, w_out, b_out):
    x = np.asarray(x, dtype=np.float32)
    w_qkv = np.asarray(w_qkv, dtype=np.float32)
    b_qkv = np.asarray(b_qkv, dtype=np.float32)
    w_out = np.asarray(w_out, dtype=np.float32)
    b_out = np.asarray(b_out, dtype=np.float32)

    if "nc" not in _CACHE:
        _CACHE["nc"] = build()
    nc = _CACHE["nc"]

    in_maps = []
    for c in range(NCORES):
        b = c // 2
        h0 = (c % 2) * HPC
        cols = slice(h0 * D, h0 * D + CPC)

        # x_r[p, kc, t] = x[b][t, kc*128+p]
        xT = x[b].T
        x_r = np.ascontiguousarray(
            xT.reshape(KC, P, T).transpose(1, 0, 2)
        ).reshape(P, KC * T)

        # wqkv_r[p, n, kc, cc] = wloc[kc*128+p, n*128+cc]
        wloc = np.concatenate(
            [w_qkv[:, cols], w_qkv[:, C:][:, cols], w_qkv[:, 2 * C:][:, cols]],
            axis=1,
        )
        wq_r = np.ascontiguousarray(
            wloc.reshape(KC, P, 12, P).transpose(1, 2, 0, 3)
        ).reshape(P, 12 * KC * P)

        bq = b_qkv[cols]
        bq_r = np.ascontiguousarray(bq.reshape(4, P).T)

        # wo_r[p, j, oc, cc] = w_out[g(j, p), oc*128+cc]
        rows = np.empty(CPC, dtype=np.int64)
        for j in range(NPAIR):
            for hh in range(2):
                base = (h0 + 2 * j + hh) * D
                rows[j * P + hh * D:j * P + (hh + 1) * D] = np.arange(
                    base, base + D
                )
        wo_perm = w_out[rows]
        wo_r = np.ascontiguousarray(
            wo_perm.reshape(NPAIR, P, 8, P).transpose(1, 0, 2, 3)
        ).reshape(P, NPAIR * 8 * P)

        # half of b_out plus this core's folded V-bias contribution
        bv_loc = b_qkv[2 * C:][cols]
        bo_eff = 0.5 * b_out + bv_loc @ w_out[cols, :]
        bo_r = np.ascontiguousarray(bo_eff.reshape(8, P).T)

        in_maps.append({
            "x_r": x_r.astype(ml_dtypes.bfloat16),
            "wqkv_r": wq_r.astype(ml_dtypes.bfloat16),
            "wo_r": wo_r.astype(ml_dtypes.bfloat16),
            "bq_r": bq_r,
            "bo_r": bo_r,
        })

    kwargs = {}
    tdir = os.environ.get("KERNEL_TRACE_DIR")
    if tdir:
        kwargs = dict(trace=True, tmpdir=tdir)
    res = run_bass_kernel_spmd(
        nc, in_maps, core_ids=list(range(NCORES)), **kwargs
    )
    _CACHE["last_results"] = res

    out = np.empty((B, T, C), dtype=np.float32)
    for c in range(NCORES):
        b = c // 2
        half = slice((c % 2) * CPC, (c % 2) * CPC + CPC)
        out[b][:, half] = res.results[c]["outT"].T.astype(np.float32)
    return out
